# revision 1
# baseline (speedup 1.0000x reference)
"""Trainium2 Bass kernel for the MGA dense-transformer block.

Reference computation (per batch n):
    qkv = depthwise3(conv1x1(x, w_qkv), w_dw)         # (3D, L)
    q,k,v per head (dh=64), l2-normalized q,k, scores = q k^T * temp,
    softmax over keys, out = attn @ v, y = conv1x1(out, w_proj)

Sharding over 8 cores: core c -> (batch n = c//2, head group g = c%2 of 4
heads).  Each core computes its 768 qkv channels, runs attention for its 4
heads, and produces a partial projection y_partial = Wp[:, cols_g] @ out_g
(512, 2048).  Host sums the two partials per batch.

Device-side layout choices:
  * qkv produced in [channel, l] layout (channel on partitions) so the
    depthwise conv is a per-partition FIR along the free dim.
  * scores are computed TRANSPOSED: S^T[lk, lq] = k_norm^T_chunk @ q_norm,
    so exp(S^T) feeds the PV matmul directly as the moving operand.
  * softmax denominator comes for free from a ones-row appended to v^T in
    the PV stationary operand (row 64 of the accumulator).
  * l2 norms via ones-vector matmuls on the PE; q/k normalization applied
    with gpsimd partition_broadcast + DVE multiply; the softmax denominator
    is broadcast across partitions with a K=1 PE matmul.
  * all matmuls run as float32r (TF32-like, 1 cycle/row at N=512).
  * build_program(repeat=R) unrolls the whole pipeline R times in one NEFF
    (used by test.py to measure per-iteration HW time via wall-clock slope).
"""

from contextlib import ExitStack

import numpy as np

import concourse.bacc as bacc
import concourse.mybir as mybir
import concourse.tile as tile
from concourse.bass_utils import run_bass_kernel_spmd

F32 = mybir.dt.float32
F32R = mybir.dt.float32r
AF = mybir.ActivationFunctionType

N, D, L, H = 4, 512, 2048, 8
DH = D // H          # 64 head dim
HPC = H // 2         # 4 heads per core
C = 3 * 256          # 768 shard qkv channels
P = 128
NLT = L // 512       # 4 query tiles
NLC = L // 128       # 16 key chunks
N_CORES = 8


def build_program(debug_dumps=False, repeat=1):
    nc = bacc.Bacc("TRN2", target_bir_lowering=False, debug=False)
    dbg = {}
    if debug_dumps:
        dbg["pre0"] = nc.dram_tensor("dbg_pre0", (P, L), F32, kind="ExternalOutput")
        dbg["dw0"] = nc.dram_tensor("dbg_dw0", (P, L), F32R, kind="ExternalOutput")
        dbg["nrm0"] = nc.dram_tensor("dbg_nrm0", (1, L), F32, kind="ExternalOutput")
        dbg["bc0"] = nc.dram_tensor("dbg_bc0", (DH, L), F32, kind="ExternalOutput")
        dbg["q0"] = nc.dram_tensor("dbg_q0", (DH, L), F32R, kind="ExternalOutput")
        dbg["vt0"] = nc.dram_tensor(
            "dbg_vt0", (P, NLC, DH + 1), F32R, kind="ExternalOutput"
        )
        dbg["es0"] = nc.dram_tensor("dbg_es0", (P, 4, 512), F32R, kind="ExternalOutput")
        dbg["outn0"] = nc.dram_tensor("dbg_outn0", (DH, L), F32R, kind="ExternalOutput")

    x_d = nc.dram_tensor("x", (D, L), F32R, kind="ExternalInput")
    wqkvT_d = nc.dram_tensor("wqkvT", (D, C), F32R, kind="ExternalInput")
    wdw_d = nc.dram_tensor("wdw", (C, 3), F32, kind="ExternalInput")
    wpT_d = nc.dram_tensor("wpT", (HPC, DH, D), F32R, kind="ExternalInput")
    temps_d = nc.dram_tensor("temps", (1, HPC), F32, kind="ExternalInput")
    # two stacked copies of eye(64) so a slice exists at base partition 0 and 64
    ident_d = nc.dram_tensor("ident", (2 * DH, DH), F32R, kind="ExternalInput")
    onesv_d = nc.dram_tensor("onesv", (P, NLC, 1), F32R, kind="ExternalInput")
    onesr_d = nc.dram_tensor("onesr", (P, DH), F32R, kind="ExternalInput")
    y_d = nc.dram_tensor("y", (D, L), F32, kind="ExternalOutput")

    with tile.TileContext(nc) as tc, ExitStack() as ctx:
        wp = ctx.enter_context(tc.tile_pool(name="w", bufs=1))
        xp = ctx.enter_context(tc.tile_pool(name="xp", bufs=4))
        esp = ctx.enter_context(tc.tile_pool(name="esp", bufs=4))
        prep = ctx.enter_context(tc.tile_pool(name="prep", bufs=2))
        dwp = ctx.enter_context(tc.tile_pool(name="dwp", bufs=6))
        tmpp = ctx.enter_context(tc.tile_pool(name="tmpp", bufs=2))
        vtp = ctx.enter_context(tc.tile_pool(name="vtp", bufs=1))
        smp = ctx.enter_context(tc.tile_pool(name="smp", bufs=1))
        psp = ctx.enter_context(tc.tile_pool(name="ps", bufs=2, space="PSUM"))
        p4p = ctx.enter_context(tc.tile_pool(name="ps4", bufs=2, space="PSUM"))
        pap = ctx.enter_context(tc.tile_pool(name="pacc", bufs=2, space="PSUM"))

        # ---- weights / constants -------------------------------------------
        wq_sb = []
        for kc in range(4):
            t = wp.tile([P, C], F32R, tag=f"wq{kc}")
            nc.gpsimd.dma_start(t[:], wqkvT_d[kc * 128:(kc + 1) * 128, :])
            wq_sb.append(t)
        wdw_sb = []
        for cc in range(6):
            t = wp.tile([P, 3], F32, tag=f"wdw{cc}")
            nc.gpsimd.dma_start(t[:], wdw_d[cc * 128:(cc + 1) * 128, :])
            wdw_sb.append(t)
        wp_sb = []
        for hl in range(HPC):
            t = wp.tile([DH, D], F32R, tag=f"wp{hl}")
            nc.gpsimd.dma_start(t[:], wpT_d[hl, :, :])
            wp_sb.append(t)
        temps_sb = wp.tile([1, HPC], F32, tag="temps")
        nc.gpsimd.dma_start(temps_sb[:], temps_d[:])
        ident_sb = wp.tile([2 * DH, DH], F32R, tag="ident")
        nc.gpsimd.dma_start(ident_sb[:], ident_d[:])
        ones_sb = wp.tile([P, 1], F32R, tag="ones")
        nc.gpsimd.dma_start(ones_sb[:], onesv_d[:, 0, :])
        onesr_sb = wp.tile([P, DH], F32R, tag="onesr")
        nc.gpsimd.dma_start(onesr_sb[:], onesr_d[:])

        x_sb = []
        dw_sb = {}
        vt_sb = {}
        outn_sb = []

        def load_x(rep):
            # x tiles and outn tiles share the "x" slots: outn takes over
            # once the conv phase has consumed x.
            x_sb.clear()
            for kc in range(4):
                t = xp.tile([P, L], F32R, tag="x", name=f"x{rep}_{kc}")
                nc.sync.dma_start(t[:], x_d[kc * 128:(kc + 1) * 128, :])
                x_sb.append(t)
            outn_sb.clear()
            outn_sb.extend(
                xp.tile([DH, L], F32R, tag="x", name=f"outn{rep}_{i}")
                for i in range(HPC)
            )

        def conv_chunk(cc):
            pre = prep.tile([P, L], F32, tag="pre", name=f"pre{cc}")
            for lt in range(NLT):
                ps = psp.tile([P, 512], F32, tag="ps", name=f"cps{cc}_{lt}")
                for kc in range(4):
                    nc.tensor.matmul(
                        ps[:],
                        wq_sb[kc][:, cc * 128:(cc + 1) * 128],
                        x_sb[kc][:, lt * 512:(lt + 1) * 512],
                        start=(kc == 0),
                        stop=(kc == 3),
                    )
                nc.scalar.copy(pre[:, lt * 512:(lt + 1) * 512], ps[:])
            dw = dwp.tile([P, L], F32R, tag="dw", name=f"dw{cc}")
            nc.scalar.mul(dw[:], pre[:], wdw_sb[cc][:, 1:2])
            nc.vector.affine_then_add(
                dw[:, 1:L], pre[:, 0:L - 1], dw[:, 1:L],
                scale=wdw_sb[cc][:, 0:1], bias=0.0,
            )
            nc.vector.affine_then_add(
                dw[:, 0:L - 1], pre[:, 1:L], dw[:, 0:L - 1],
                scale=wdw_sb[cc][:, 2:3], bias=0.0,
            )
            dw_sb[cc] = dw
            if debug_dumps and cc == 0:
                nc.sync.dma_start(dbg["pre0"][:], pre[:])
                nc.sync.dma_start(dbg["dw0"][:], dw[:])

        def ch_slice(base, hl):
            c0 = base + DH * hl
            return dw_sb[c0 // 128][c0 % 128:c0 % 128 + DH, :]

        def normalize(hl, is_q):
            # l2-normalize q or k of head hl in place (temp folded into q)
            s = ch_slice(0 if is_q else 256, hl)
            b = DH * (hl % 2)  # base partition of this head's slice
            sq = tmpp.tile([P, L], F32R, tag="sqbc", bufs=2, name=f"sq{hl}{is_q}")
            nc.vector.tensor_mul(sq[b:b + DH, :], s[:], s[:])
            nrm = smp.tile([1, L], F32, tag="nrm", bufs=1, name=f"nrm{hl}{is_q}")
            for lt in range(NLT):
                ps = psp.tile([1, 512], F32, tag="ps", name=f"nps{hl}{is_q}{lt}")
                nc.tensor.matmul(
                    ps[:],
                    ones_sb[b:b + DH, :],
                    sq[b:b + DH, lt * 512:(lt + 1) * 512],
                    start=True,
                    stop=True,
                )
                nc.scalar.activation(nrm[:, lt * 512:(lt + 1) * 512], ps[:], AF.Sqrt)
            nc.vector.reciprocal_approx_fast(nrm[:], nrm[:])
            if is_q:
                nc.vector.tensor_scalar_mul(nrm[:], nrm[:], temps_sb[:, hl:hl + 1])
            bc = tmpp.tile([P, L], F32, tag="sqbc", bufs=2, name=f"bc{hl}{is_q}")
            nc.gpsimd.partition_broadcast(bc[:], nrm[:])
            if debug_dumps and hl == 0 and is_q:
                nc.sync.dma_start(dbg["nrm0"][:], nrm[:])
                nc.sync.dma_start(dbg["bc0"][:], bc[b:b + DH, :])
            nc.vector.tensor_mul(s[:], s[:], bc[b:b + DH, :])
            if debug_dumps and hl == 0 and is_q:
                nc.sync.dma_start(dbg["q0"][:], s[:])

        def build_vt(hl):
            # v^T plus a ones row for the softmax denominator
            v = ch_slice(512, hl)
            vt = vtp.tile([P, NLC, DH + 1], F32R, tag=f"vt{hl}", name=f"vt{hl}")
            nc.gpsimd.dma_start(vt[:, :, DH:DH + 1], onesv_d[:])
            vbase = (512 + DH * hl) % 128  # base partition of the v slice (0 or 64)
            ident = ident_sb[vbase:vbase + DH, :]
            # 4 transposes form one accumulation group in a single PSUM bank
            for lg in range(NLC // 4):
                ps = psp.tile([P, 4, DH], F32R, tag="ps", name=f"tps{hl}_{lg}")
                for j in range(4):
                    lc = 4 * lg + j
                    nc.tensor.matmul(
                        ps[:, j, :], v[:, lc * 128:(lc + 1) * 128], ident,
                        is_transpose=True, start=(j == 0), stop=(j == 3),
                    )
                nc.vector.tensor_copy(vt[:, 4 * lg:4 * lg + 4, 0:DH], ps[:])
            vt_sb[hl] = vt
            if debug_dumps and hl == 0:
                nc.sync.dma_start(dbg["vt0"][:], vt[:])

        def attention(hl):
            q = ch_slice(0, hl)
            k = ch_slice(256, hl)
            vt = vt_sb[hl]
            for lt in range(NLT):
                stripes = [
                    esp.tile([P, 4, 512], F32R, tag="es", name=f"es_{hl}_{lt}_{i}")
                    for i in range(4)
                ]
                po = pap.tile([P, 512], F32, tag="pacc")
                for g in range(NLC // 2):
                    ps4 = p4p.tile([P, 2, 512], F32, tag="ps4")
                    for j in range(2):
                        lc = 2 * g + j
                        nc.tensor.matmul(
                            ps4[:, j, :],
                            k[:, lc * 128:(lc + 1) * 128],
                            q[:, lt * 512:(lt + 1) * 512],
                            start=True,
                            stop=True,
                        )
                    st = stripes[g // 2]
                    nc.scalar.activation(
                        st[:, 2 * (g % 2):2 * (g % 2) + 2, :], ps4[:], AF.Exp
                    )
                    for j in range(2):
                        lc = 2 * g + j
                        nc.tensor.matmul(
                            po[0:DH + 1, :],
                            vt[:, lc, :],
                            st[:, lc % 4, :],
                            start=(lc == 0),
                            stop=(lc == NLC - 1),
                        )
                if debug_dumps and hl == 0 and lt == 0:
                    nc.sync.dma_start(dbg["es0"][:], stripes[0][:])
                rec = smp.tile([P, 512], F32R, tag="rec", bufs=2)
                with nc.allow_low_precision(reason="1/denom feeds an f32r matmul"):
                    nc.vector.reciprocal(rec[DH:DH + 1, :], po[DH:DH + 1, :])
                # broadcast 1/denom across 64 partitions via a K=1 matmul
                bcb = p4p.tile([DH, 512], F32, tag="ps4")
                nc.tensor.matmul(
                    bcb[:], onesr_sb[DH:DH + 1, :], rec[DH:DH + 1, :],
                    start=True, stop=True,
                )
                bcd = smp.tile([DH, 512], F32, tag="bcd", bufs=2)
                nc.vector.tensor_copy(bcd[:], bcb[:])
                dst = outn_sb[hl][:, lt * 512:(lt + 1) * 512]
                nc.vector.tensor_mul(dst, po[0:DH, :], bcd[:])

        for rep in range(repeat):
            load_x(rep)
            for cc in range(6):
                conv_chunk(cc)
            for hl in range(HPC):
                normalize(hl, True)
                normalize(hl, False)
                build_vt(hl)
            for hl in range(HPC):
                attention(hl)

            if debug_dumps:
                nc.sync.dma_start(dbg["outn0"][:], outn_sb[0][:])

            # ---- output projection (partial over this core's channels) ------
            for oc in range(4):
                ysb = smp.tile([P, L], F32, tag="ysb", bufs=1, name=f"ysb{rep}_{oc}")
                for lt in range(NLT):
                    ps = psp.tile([P, 512], F32, tag="ps", name=f"yps{rep}_{oc}{lt}")
                    for hl in range(HPC):
                        nc.tensor.matmul(
                            ps[:],
                            wp_sb[hl][:, oc * 128:(oc + 1) * 128],
                            outn_sb[hl][:, lt * 512:(lt + 1) * 512],
                            start=(hl == 0),
                            stop=(hl == HPC - 1),
                        )
                    nc.scalar.copy(ysb[:, lt * 512:(lt + 1) * 512], ps[:])
                nc.sync.dma_start(y_d[oc * 128:(oc + 1) * 128, :], ysb[:])

    nc.compile()
    return nc


def make_in_maps(x, w_qkv, w_dw, w_proj, temperature):
    x = np.asarray(x, dtype=np.float32)
    w_qkv = np.asarray(w_qkv, dtype=np.float32)
    w_dw = np.asarray(w_dw, dtype=np.float32)
    w_proj = np.asarray(w_proj, dtype=np.float32)
    temperature = np.asarray(temperature, dtype=np.float32)
    in_maps = []
    for c in range(N_CORES):
        n, g = c // 2, c % 2
        rows = np.concatenate(
            [256 * g + np.arange(256) + off for off in (0, 512, 1024)]
        )
        in_maps.append(
            {
                "x": np.ascontiguousarray(x[n]),
                "wqkvT": np.ascontiguousarray(w_qkv[rows, :, 0].T),
                "wdw": np.ascontiguousarray(w_dw[rows, 0, :]),
                "wpT": np.ascontiguousarray(
                    w_proj[:, 256 * g:256 * g + 256, 0].T.reshape(HPC, DH, D)
                ),
                "temps": np.ascontiguousarray(
                    temperature[0, HPC * g:HPC * g + HPC, 0, 0][None, :]
                ),
                "ident": np.vstack([np.eye(DH, dtype=np.float32)] * 2),
                "onesv": np.ones((P, NLC, 1), dtype=np.float32),
                "onesr": np.ones((P, DH), dtype=np.float32),
            }
        )
    return in_maps


_PROGRAM = None


def _get_program():
    global _PROGRAM
    if _PROGRAM is None:
        _PROGRAM = build_program()
    return _PROGRAM


def kernel(x, w_qkv, w_dw, w_proj, temperature):
    prog = _get_program()
    in_maps = make_in_maps(x, w_qkv, w_dw, w_proj, temperature)
    res = run_bass_kernel_spmd(prog, in_maps, list(range(N_CORES)))
    y = np.empty((N, D, L), np.float32)
    for n in range(N):
        y[n] = res.results[2 * n]["y"] + res.results[2 * n + 1]["y"]
    return y


if __name__ == "__main__":
    prog = build_program()
    print("program built ok")

